# revision 14
# baseline (speedup 1.0000x reference)
"""Bahdanau additive attention kernel for 8 Trainium2 NeuronCores.

Reference computation (per batch b):
  q = query @ W1                  [TQ, U]
  k = value @ W2                  [TV, U]
  scores[i,j] = sum_u scale[u] * tanh(q[i,u] + k[j,u])
  attn = softmax(scores, axis=-1)
  ctx = attn @ value

Sharding: the B*TQ = 2048 query rows split into 8 chunks of 256; core c
handles batch c//4, query rows (c%4)*256 .. +256.  Each core gets its
query chunk plus the full value/W1/W2/scale (data-parallel, no
collectives).

Per-core dataflow (U = 128 = partition count):
  - load query/value/W tiles (DMAs spread over three queues), convert to
    fp16 on GpSimd, PE-transpose the fp16 copies to get d-major layouts
    (fp16 keeps the PE off its 2-pass fp32 LOW_HIGH weight-load path;
    fp16 rounding of the projection inputs costs ~5e-4 relative)
  - kprojT [U, TV] and qprojT [U, QCH] via fp16 PE matmuls (fp32 psum)
  - per query i: ACT computes t = tanh(kprojT + qprojT[:, i]) in one
    [128, 1024] instruction (per-partition bias), output cast to fp16;
    8 PE matmuls with t as the stationary operand and scale [128,1]
    moving produce the score column for each 128-key tile
  - per 64-query half-tile: PE-transpose scoresT back (fp32, separate
    psum/sbuf tiles per half so the first half's softmax prep overlaps
    the tanh stream), batched softmax (DVE max -> ACT exp with
    accumulator -> DVE reciprocal -> DVE scale), fp16 attn copy for the
    context matmul against fp16 value tiles.  Both qtiles' tanh/score
    loops are emitted before any softmax so the ACT stream never stalls.
"""

from contextlib import ExitStack

import numpy as np

from concourse import bacc, bass, masks, mybir
import concourse.tile as tile
from concourse.bass_utils import run_bass_kernel_spmd

F32 = mybir.dt.float32
F16 = mybir.dt.float16
AF = mybir.ActivationFunctionType
ALU = mybir.AluOpType
AX = mybir.AxisListType

B, TQ, TV, D, U = 2, 1024, 1024, 512, 128
NCORES = 8
QCH = (B * TQ) // NCORES  # 256 query rows per core
NQT = QCH // 128          # 2 query tiles per core
NVT = TV // 128           # 8 value tiles
NDT = D // 128            # 4 d tiles


def build_program() -> bass.Bass:
    nc = bacc.Bacc(None, target_bir_lowering=False)
    q_in = nc.declare_dram_parameter("q", [QCH, D], F32, isOutput=False)
    v_in = nc.declare_dram_parameter("v", [TV, D], F32, isOutput=False)
    w1_in = nc.declare_dram_parameter("w1", [D, U], F32, isOutput=False)
    w2_in = nc.declare_dram_parameter("w2", [D, U], F32, isOutput=False)
    s_in = nc.declare_dram_parameter("s", [U, 1], F32, isOutput=False)
    attn_out = nc.declare_dram_parameter("attn", [QCH, TV], F32, isOutput=True)
    ctx_out = nc.declare_dram_parameter("ctx", [QCH, D], F32, isOutput=True)

    with tile.TileContext(nc) as tc, ExitStack() as ctx:
        const = ctx.enter_context(tc.tile_pool(name="const", bufs=1))
        sb = ctx.enter_context(tc.tile_pool(name="sb", bufs=1))
        tpool = ctx.enter_context(tc.tile_pool(name="tpool", bufs=4))
        qt_pool = ctx.enter_context(tc.tile_pool(name="qt_pool", bufs=2))
        half_pool = ctx.enter_context(tc.tile_pool(name="half_pool", bufs=4))
        small = ctx.enter_context(tc.tile_pool(name="small", bufs=8))
        # PSUM: scoresT half tiles are 1 bank each, 4 alive at once;
        # everything else shares one-bank slots.
        ps_sc = ctx.enter_context(tc.tile_pool(name="ps_sc", bufs=4, space="PSUM"))
        ps_misc = ctx.enter_context(tc.tile_pool(name="ps_misc", bufs=2, space="PSUM"))

        identity = const.tile([128, 128], F32)
        masks.make_identity(nc, identity[:])
        identity16 = const.tile([128, 128], F16)
        masks.make_identity(nc, identity16[:])

        s_sb = const.tile([U, 1], F32)
        nc.scalar.dma_start(out=s_sb[:], in_=s_in[:])
        # fp16 copy of scale: the score matmuls run in fp16 (fp32 stationary
        # operands force a 2-pass LOW_HIGH weight load on the PE).
        s16_sb = const.tile([U, 1], F16)
        nc.vector.tensor_copy(s16_sb[:], s_sb[:])

        # DMAs: value on sync, weights on gpsimd, query/scale on vector.
        w1_sb = sb.tile([128, NDT, U], F32)
        w2_sb = sb.tile([128, NDT, U], F32)
        for d in range(NDT):
            nc.gpsimd.dma_start(out=w2_sb[:, d, :], in_=w2_in[d * 128:(d + 1) * 128, :])
        for d in range(NDT):
            nc.gpsimd.dma_start(out=w1_sb[:, d, :], in_=w1_in[d * 128:(d + 1) * 128, :])
        value_sb = sb.tile([128, NVT, D], F32)
        for t in range(NVT):
            nc.sync.dma_start(out=value_sb[:, t, :], in_=v_in[t * 128:(t + 1) * 128, :])
        query_sb = sb.tile([128, NQT, D], F32)
        for t in range(NQT):
            nc.scalar.dma_start(out=query_sb[:, t, :], in_=q_in[t * 128:(t + 1) * 128, :])

        # fp16 working copies (GpSimd: SBUF->SBUF casts, keeps DVE free for
        # the psum evacuation copies)
        w2_16_sb = sb.tile([128, NDT, U], F16)
        for d in range(NDT):
            nc.gpsimd.tensor_copy(w2_16_sb[:, d, :], w2_sb[:, d, :])
        value16_sb = sb.tile([128, NVT, D], F16)
        for t in range(NVT):
            nc.gpsimd.tensor_copy(value16_sb[:, t, :], value_sb[:, t, :])
        query16_sb = sb.tile([128, NQT, D], F16)
        for t in range(NQT):
            nc.gpsimd.tensor_copy(query16_sb[:, t, :], query_sb[:, t, :])
        w1_16_sb = sb.tile([128, NDT, U], F16)
        for d in range(NDT):
            nc.gpsimd.tensor_copy(w1_16_sb[:, d, :], w1_sb[:, d, :])

        # valueT16[p, d, j] = value[j, d*128+p]; one batched psum tile +
        # one DVE copy per value tile
        valueT16_sb = sb.tile([128, NDT, TV], F16)
        for t in range(NVT):
            pst = ps_misc.tile([128, NDT, 128], F16, tag="ps_misc")
            for d in range(NDT):
                nc.tensor.transpose(pst[:, d, :], value16_sb[:, t, d * 128:(d + 1) * 128],
                                    identity16[:])
            nc.vector.tensor_copy(valueT16_sb[:, :, t * 128:(t + 1) * 128], pst[:])
        # queryT16[p, d, i] = query[i, d*128+p]
        queryT16_sb = sb.tile([128, NDT, QCH], F16)
        for t in range(NQT):
            pst = ps_misc.tile([128, NDT, 128], F16, tag="ps_misc")
            for d in range(NDT):
                nc.tensor.transpose(pst[:, d, :], query16_sb[:, t, d * 128:(d + 1) * 128],
                                    identity16[:])
            nc.vector.tensor_copy(queryT16_sb[:, :, t * 128:(t + 1) * 128], pst[:])

        # qprojT[u, i] = sum_d W1[d, u] * query[i, d]   (fp32 accumulate)
        qprojT_sb = sb.tile([U, QCH], F32)
        psq = ps_misc.tile([U, QCH], F32, tag="ps_misc")
        for d in range(NDT):
            nc.tensor.matmul(psq[:], w1_16_sb[:, d, :], queryT16_sb[:, d, :],
                             start=(d == 0), stop=(d == NDT - 1))
        nc.vector.tensor_copy(qprojT_sb[:], psq[:])

        # kprojT[u, j] = sum_d W2[d, u] * value[j, d]
        kprojT_sb = sb.tile([U, TV], F32)
        for h in range(2):
            psk = ps_misc.tile([U, 512], F32, tag="ps_misc")
            for d in range(NDT):
                nc.tensor.matmul(psk[:], w2_16_sb[:, d, :],
                                 valueT16_sb[:, d, h * 512:(h + 1) * 512],
                                 start=(d == 0), stop=(d == NDT - 1))
            nc.vector.tensor_copy(kprojT_sb[:, h * 512:(h + 1) * 512], psk[:])

        # ---- tanh + score matvecs for BOTH qtiles (uninterrupted ACT stream)
        # scoresT_ps[qt][h][m, vt, c] = scores[qt*128 + h*64 + c, vt*128 + m]
        scoresT_ps = [[None, None] for _ in range(NQT)]
        for qt in range(NQT):
            for qi in range(128):
                h, c = qi // 64, qi % 64
                if c == 0:
                    sc_ps_half = ps_sc.tile([128, NVT, 64], F32, tag="sc_ps_half")
                    scoresT_ps[qt][h] = sc_ps_half
                i = qt * 128 + qi
                t_t = tpool.tile([U, TV], F16)
                nc.scalar.activation(out=t_t[:], in_=kprojT_sb[:], func=AF.Tanh,
                                     bias=qprojT_sb[:, i:i + 1], scale=1.0)
                for vt in range(NVT):
                    nc.tensor.matmul(scoresT_ps[qt][h][:, vt, c:c + 1],
                                     t_t[:, vt * 128:(vt + 1) * 128], s16_sb[:],
                                     start=True, stop=True)

        # ---- softmax + context, fully per 64-query half-block (all tiles at
        # partition base 0 so the PE transpose partition-0 rule is satisfied
        # and the first half's chain overlaps the remaining tanh stream)
        for qt in range(NQT):
            for h in range(2):
                row0 = qt * 128 + h * 64
                scoresT_sb = half_pool.tile([128, NVT, 64], F32, tag="scoresT")
                nc.vector.tensor_copy(scoresT_sb[:], scoresT_ps[qt][h][:])
                scores_sb = half_pool.tile([64, TV], F32, tag="scores")
                for vt in range(NVT):
                    pst = ps_misc.tile([64, 128], F32, tag="ps_tr")
                    nc.tensor.transpose(pst[:], scoresT_sb[:, vt, :], identity[:])
                    nc.vector.tensor_copy(scores_sb[:, vt * 128:(vt + 1) * 128], pst[:])

                neg_max = small.tile([64, 1], F32, tag="small")
                nc.vector.tensor_reduce(out=neg_max[:], in_=scores_sb[:],
                                        axis=AX.X, op=ALU.max, negate=True)
                exp_sb = half_pool.tile([64, TV], F32, tag="exp")
                sums = small.tile([64, 1], F32, tag="small")
                nc.scalar.activation(out=exp_sb[:], in_=scores_sb[:],
                                     func=AF.Exp, bias=neg_max[:],
                                     accum_out=sums[:])
                recip = small.tile([64, 1], F32, tag="small")
                nc.vector.reciprocal(recip[:], sums[:])
                attn_sb = half_pool.tile([64, TV], F32, tag="attn")
                nc.vector.tensor_scalar_mul(attn_sb[:], exp_sb[:], recip[:])
                nc.sync.dma_start(out=attn_out[row0:row0 + 64, :], in_=attn_sb[:])

                # fp16 attn copy for the context matmul
                attn16_sb = half_pool.tile([64, TV], F16, tag="attn16")
                nc.vector.tensor_scalar_mul(attn16_sb[:], exp_sb[:], recip[:])
                attnT16_sb = half_pool.tile([128, NVT, 64], F16, tag="attnT")
                for vt in range(NVT):
                    pst = ps_misc.tile([128, 64], F16, tag="ps_tr")
                    nc.tensor.transpose(pst[:], attn16_sb[:, vt * 128:(vt + 1) * 128],
                                        identity16[0:64, 0:64])
                    nc.vector.tensor_copy(attnT16_sb[:, vt, :], pst[:])

                ctx_ps = ps_misc.tile([64, D], F32, tag="ps_tr")
                for vt in range(NVT):
                    nc.tensor.matmul(ctx_ps[:], attnT16_sb[:, vt, :],
                                     value16_sb[:, vt, :],
                                     start=(vt == 0), stop=(vt == NVT - 1))
                ctx_sb = half_pool.tile([64, D], F32, tag="ctx")
                nc.vector.tensor_copy(ctx_sb[:], ctx_ps[:])
                nc.sync.dma_start(out=ctx_out[row0:row0 + 64, :], in_=ctx_sb[:])

    nc.finalize()
    return nc


_program_cache: dict[str, bass.Bass] = {}


def _get_program() -> bass.Bass:
    if "nc" not in _program_cache:
        _program_cache["nc"] = build_program()
    return _program_cache["nc"]


def make_in_maps(query, value, W1, W2, scale):
    in_maps = []
    for c in range(NCORES):
        b = c // (NCORES // B)
        qc = c % (NCORES // B)
        in_maps.append({
            "q": np.ascontiguousarray(query[b, qc * QCH:(qc + 1) * QCH, :], dtype=np.float32),
            "v": np.ascontiguousarray(value[b], dtype=np.float32),
            "w1": np.ascontiguousarray(W1, dtype=np.float32),
            "w2": np.ascontiguousarray(W2, dtype=np.float32),
            "s": np.ascontiguousarray(np.asarray(scale).reshape(U, 1), dtype=np.float32),
        })
    return in_maps


def assemble(results):
    ctx_full = np.empty((B, TQ, D), dtype=np.float32)
    attn_full = np.empty((B, TQ, TV), dtype=np.float32)
    for c in range(NCORES):
        b = c // (NCORES // B)
        qc = c % (NCORES // B)
        ctx_full[b, qc * QCH:(qc + 1) * QCH, :] = results[c]["ctx"]
        attn_full[b, qc * QCH:(qc + 1) * QCH, :] = results[c]["attn"]
    return ctx_full, attn_full


def kernel(query, value, W1, W2, scale):
    nc = _get_program()
    in_maps = make_in_maps(query, value, W1, W2, scale)
    res = run_bass_kernel_spmd(nc, in_maps, list(range(NCORES))).results
    return assemble(res)


# revision 16
# speedup vs baseline: 1.1213x; 1.1213x over previous
"""Bahdanau additive attention kernel for 8 Trainium2 NeuronCores.

Reference computation (per batch b):
  q = query @ W1                  [TQ, U]
  k = value @ W2                  [TV, U]
  scores[i,j] = sum_u scale[u] * tanh(q[i,u] + k[j,u])
  attn = softmax(scores, axis=-1)
  ctx = attn @ value

Sharding: the B*TQ = 2048 query rows split into 8 chunks of 256; core c
handles batch c//4, query rows (c%4)*256 .. +256.  Each core gets its
query chunk plus the full value/W1/W2/scale (data-parallel, no
collectives).

Per-core dataflow (U = 128 = partition count):
  - load query/value/W tiles (DMAs spread over three queues), convert to
    fp16 on DVE, PE-transpose the fp16 copies to get d-major layouts
    (fp16 keeps the PE off its 2-pass fp32 LOW_HIGH weight-load path;
    fp16 rounding of the projection inputs costs ~5e-4 relative)
  - kprojT [U, TV] and qprojT [U, QCH] via fp16 PE matmuls (fp32 psum)
  - per query i: ACT computes t = tanh(kprojT + qprojT[:, i]) in one
    [128, 1024] instruction (per-partition bias), output cast to fp16;
    8 PE matmuls with t as the stationary operand and scale [128,1]
    moving produce the score column for each 128-key tile
  - per 64-query half-tile: PE-transpose scoresT back (fp32, separate
    psum/sbuf tiles per half so the first half's softmax prep overlaps
    the tanh stream), batched softmax (DVE max -> ACT exp with
    accumulator -> DVE reciprocal -> DVE scale), fp16 attn copy for the
    context matmul against fp16 value tiles.  Both qtiles' tanh/score
    loops are emitted before any softmax so the ACT stream never stalls.
"""

from contextlib import ExitStack

import numpy as np

from concourse import bacc, bass, masks, mybir
import concourse.tile as tile
from concourse.bass_utils import run_bass_kernel_spmd

F32 = mybir.dt.float32
F16 = mybir.dt.float16
AF = mybir.ActivationFunctionType
ALU = mybir.AluOpType
AX = mybir.AxisListType

B, TQ, TV, D, U = 2, 1024, 1024, 512, 128
NCORES = 8
QCH = (B * TQ) // NCORES  # 256 query rows per core
NQT = QCH // 128          # 2 query tiles per core
NVT = TV // 128           # 8 value tiles
NDT = D // 128            # 4 d tiles


def build_program() -> bass.Bass:
    nc = bacc.Bacc(None, target_bir_lowering=False)
    q_in = nc.declare_dram_parameter("q", [QCH, D], F32, isOutput=False)
    v_in = nc.declare_dram_parameter("v", [TV, D], F32, isOutput=False)
    w1_in = nc.declare_dram_parameter("w1", [D, U], F32, isOutput=False)
    w2_in = nc.declare_dram_parameter("w2", [D, U], F32, isOutput=False)
    s_in = nc.declare_dram_parameter("s", [U, 1], F32, isOutput=False)
    attn_out = nc.declare_dram_parameter("attn", [QCH, TV], F32, isOutput=True)
    ctx_out = nc.declare_dram_parameter("ctx", [QCH, D], F32, isOutput=True)

    with tile.TileContext(nc) as tc, ExitStack() as ctx:
        const = ctx.enter_context(tc.tile_pool(name="const", bufs=1))
        sb = ctx.enter_context(tc.tile_pool(name="sb", bufs=1))
        tpool = ctx.enter_context(tc.tile_pool(name="tpool", bufs=4))
        qt_pool = ctx.enter_context(tc.tile_pool(name="qt_pool", bufs=2))
        half_pool = ctx.enter_context(tc.tile_pool(name="half_pool", bufs=4))
        small = ctx.enter_context(tc.tile_pool(name="small", bufs=8))
        # PSUM: scoresT half tiles are 1 bank each, 4 alive at once;
        # everything else shares one-bank slots.
        ps_sc = ctx.enter_context(tc.tile_pool(name="ps_sc", bufs=2, space="PSUM"))
        ps_misc = ctx.enter_context(tc.tile_pool(name="ps_misc", bufs=2, space="PSUM"))
        ps_tr = ctx.enter_context(tc.tile_pool(name="ps_tr", bufs=4, space="PSUM"))

        identity = const.tile([128, 128], F32)
        masks.make_identity(nc, identity[:])
        identity16 = const.tile([128, 128], F16)
        masks.make_identity(nc, identity16[:])

        s_sb = const.tile([U, 1], F32)
        nc.scalar.dma_start(out=s_sb[:], in_=s_in[:])
        # fp16 copy of scale: the score matmuls run in fp16 (fp32 stationary
        # operands force a 2-pass LOW_HIGH weight load on the PE).
        s16_sb = const.tile([U, 1], F16)
        nc.vector.tensor_copy(s16_sb[:], s_sb[:])

        # DMAs: value on sync, weights on gpsimd, query/scale on vector.
        w1_sb = sb.tile([128, NDT, U], F32)
        w2_sb = sb.tile([128, NDT, U], F32)
        for d in range(NDT):
            nc.gpsimd.dma_start(out=w2_sb[:, d, :], in_=w2_in[d * 128:(d + 1) * 128, :])
        for d in range(NDT):
            nc.gpsimd.dma_start(out=w1_sb[:, d, :], in_=w1_in[d * 128:(d + 1) * 128, :])
        value_sb = sb.tile([128, NVT, D], F32)
        for t in range(NVT):
            nc.sync.dma_start(out=value_sb[:, t, :], in_=v_in[t * 128:(t + 1) * 128, :])
        query_sb = sb.tile([128, NQT, D], F32)
        for t in range(NQT):
            nc.scalar.dma_start(out=query_sb[:, t, :], in_=q_in[t * 128:(t + 1) * 128, :])

        # fp16 working copies
        w2_16_sb = sb.tile([128, NDT, U], F16)
        for d in range(NDT):
            nc.vector.tensor_copy(w2_16_sb[:, d, :], w2_sb[:, d, :])
        value16_sb = sb.tile([128, NVT, D], F16)
        for t in range(NVT):
            nc.vector.tensor_copy(value16_sb[:, t, :], value_sb[:, t, :])
        query16_sb = sb.tile([128, NQT, D], F16)
        for t in range(NQT):
            nc.vector.tensor_copy(query16_sb[:, t, :], query_sb[:, t, :])
        w1_16_sb = sb.tile([128, NDT, U], F16)
        for d in range(NDT):
            nc.vector.tensor_copy(w1_16_sb[:, d, :], w1_sb[:, d, :])

        # valueT16[p, d, j] = value[j, d*128+p]; one batched psum tile +
        # one DVE copy per value tile
        valueT16_sb = sb.tile([128, NDT, TV], F16)
        for t in range(NVT):
            pst = ps_misc.tile([128, NDT, 128], F16, tag="ps_misc")
            for d in range(NDT):
                nc.tensor.transpose(pst[:, d, :], value16_sb[:, t, d * 128:(d + 1) * 128],
                                    identity16[:])
            nc.vector.tensor_copy(valueT16_sb[:, :, t * 128:(t + 1) * 128], pst[:])
        # queryT16[p, d, i] = query[i, d*128+p]
        queryT16_sb = sb.tile([128, NDT, QCH], F16)
        for t in range(NQT):
            pst = ps_misc.tile([128, NDT, 128], F16, tag="ps_misc")
            for d in range(NDT):
                nc.tensor.transpose(pst[:, d, :], query16_sb[:, t, d * 128:(d + 1) * 128],
                                    identity16[:])
            nc.vector.tensor_copy(queryT16_sb[:, :, t * 128:(t + 1) * 128], pst[:])

        # qprojT[u, i] = sum_d W1[d, u] * query[i, d]   (fp32 accumulate)
        qprojT_sb = sb.tile([U, QCH], F32)
        psq = ps_misc.tile([U, QCH], F32, tag="ps_misc")
        for d in range(NDT):
            nc.tensor.matmul(psq[:], w1_16_sb[:, d, :], queryT16_sb[:, d, :],
                             start=(d == 0), stop=(d == NDT - 1))
        nc.vector.tensor_copy(qprojT_sb[:], psq[:])

        # kprojT[u, j] = sum_d W2[d, u] * value[j, d]
        kprojT_sb = sb.tile([U, TV], F32)
        for h in range(2):
            psk = ps_misc.tile([U, 512], F32, tag="ps_misc")
            for d in range(NDT):
                nc.tensor.matmul(psk[:], w2_16_sb[:, d, :],
                                 valueT16_sb[:, d, h * 512:(h + 1) * 512],
                                 start=(d == 0), stop=(d == NDT - 1))
            nc.vector.tensor_copy(kprojT_sb[:, h * 512:(h + 1) * 512], psk[:])

        # ---- main stream, emitted per 64-query half-block with a 1-half
        # lag: PE executes its queue in order, so each half's softmax-prep
        # transposes are queued right after the NEXT half's score matmuls.
        # That way they run while ACT is still on the next half's tanh
        # stream, and each exp's dependency chain is already done when ACT
        # reaches it.  Only the last half's chain is exposed at the end.

        def emit_half_scores(qt, h):
            # sc_ps[m, vt, c] = scores[qt*128 + h*64 + c, vt*128 + m]
            sc_ps = ps_sc.tile([128, NVT, 64], F32, tag="sc_ps_half")
            for c in range(64):
                i = qt * 128 + h * 64 + c
                t_t = tpool.tile([U, TV], F16, tag="t_t")
                nc.scalar.activation(out=t_t[:], in_=kprojT_sb[:], func=AF.Tanh,
                                     bias=qprojT_sb[:, i:i + 1], scale=1.0)
                for vt in range(NVT):
                    nc.tensor.matmul(sc_ps[:, vt, c:c + 1],
                                     t_t[:, vt * 128:(vt + 1) * 128], s16_sb[:],
                                     start=True, stop=True)
            return sc_ps

        def emit_half_softmax(qt, h, sc_ps):
            row0 = qt * 128 + h * 64
            scoresT_sb = half_pool.tile([128, NVT, 64], F32, tag="scoresT")
            nc.vector.tensor_copy(scoresT_sb[:], sc_ps[:])
            scores_sb = half_pool.tile([64, TV], F32, tag="scores")
            for vt in range(NVT):
                pst = ps_tr.tile([64, 128], F32, tag="ps_tr")
                nc.tensor.transpose(pst[:], scoresT_sb[:, vt, :], identity[:])
                nc.vector.tensor_copy(scores_sb[:, vt * 128:(vt + 1) * 128], pst[:])

            neg_max = small.tile([64, 1], F32, tag="small")
            nc.vector.tensor_reduce(out=neg_max[:], in_=scores_sb[:],
                                    axis=AX.X, op=ALU.max, negate=True)
            exp_sb = half_pool.tile([64, TV], F32, tag="exp")
            sums = small.tile([64, 1], F32, tag="small")
            nc.scalar.activation(out=exp_sb[:], in_=scores_sb[:],
                                 func=AF.Exp, bias=neg_max[:],
                                 accum_out=sums[:])
            recip = small.tile([64, 1], F32, tag="small")
            nc.vector.reciprocal(recip[:], sums[:])
            attn_sb = half_pool.tile([64, TV], F32, tag="attn")
            nc.vector.tensor_scalar_mul(attn_sb[:], exp_sb[:], recip[:])
            nc.sync.dma_start(out=attn_out[row0:row0 + 64, :], in_=attn_sb[:])

            # fp16 attn copy for the context matmul
            attn16_sb = half_pool.tile([64, TV], F16, tag="attn16")
            nc.vector.tensor_scalar_mul(attn16_sb[:], exp_sb[:], recip[:])
            attnT16_sb = half_pool.tile([128, NVT, 64], F16, tag="attnT")
            for vt in range(NVT):
                pst = ps_tr.tile([128, 64], F16, tag="ps_tr")
                nc.tensor.transpose(pst[:], attn16_sb[:, vt * 128:(vt + 1) * 128],
                                    identity16[0:64, 0:64])
                nc.vector.tensor_copy(attnT16_sb[:, vt, :], pst[:])

            ctx_ps = ps_tr.tile([64, D], F32, tag="ps_tr")
            for vt in range(NVT):
                nc.tensor.matmul(ctx_ps[:], attnT16_sb[:, vt, :],
                                 value16_sb[:, vt, :],
                                 start=(vt == 0), stop=(vt == NVT - 1))
            ctx_sb = half_pool.tile([64, D], F32, tag="ctx")
            nc.vector.tensor_copy(ctx_sb[:], ctx_ps[:])
            nc.sync.dma_start(out=ctx_out[row0:row0 + 64, :], in_=ctx_sb[:])

        halves = [(qt, h) for qt in range(NQT) for h in range(2)]
        prev = None
        for qt, h in halves:
            sc_ps = emit_half_scores(qt, h)
            if prev is not None:
                emit_half_softmax(*prev)
            prev = (qt, h, sc_ps)
        emit_half_softmax(*prev)

    nc.finalize()
    return nc


_program_cache: dict[str, bass.Bass] = {}


def _get_program() -> bass.Bass:
    if "nc" not in _program_cache:
        _program_cache["nc"] = build_program()
    return _program_cache["nc"]


def make_in_maps(query, value, W1, W2, scale):
    in_maps = []
    for c in range(NCORES):
        b = c // (NCORES // B)
        qc = c % (NCORES // B)
        in_maps.append({
            "q": np.ascontiguousarray(query[b, qc * QCH:(qc + 1) * QCH, :], dtype=np.float32),
            "v": np.ascontiguousarray(value[b], dtype=np.float32),
            "w1": np.ascontiguousarray(W1, dtype=np.float32),
            "w2": np.ascontiguousarray(W2, dtype=np.float32),
            "s": np.ascontiguousarray(np.asarray(scale).reshape(U, 1), dtype=np.float32),
        })
    return in_maps


def assemble(results):
    ctx_full = np.empty((B, TQ, D), dtype=np.float32)
    attn_full = np.empty((B, TQ, TV), dtype=np.float32)
    for c in range(NCORES):
        b = c // (NCORES // B)
        qc = c % (NCORES // B)
        ctx_full[b, qc * QCH:(qc + 1) * QCH, :] = results[c]["ctx"]
        attn_full[b, qc * QCH:(qc + 1) * QCH, :] = results[c]["attn"]
    return ctx_full, attn_full


def kernel(query, value, W1, W2, scale):
    nc = _get_program()
    in_maps = make_in_maps(query, value, W1, W2, scale)
    res = run_bass_kernel_spmd(nc, in_maps, list(range(NCORES))).results
    return assemble(res)


# revision 17
# speedup vs baseline: 1.1408x; 1.0174x over previous
"""Bahdanau additive attention kernel for 8 Trainium2 NeuronCores.

Reference computation (per batch b):
  q = query @ W1                  [TQ, U]
  k = value @ W2                  [TV, U]
  scores[i,j] = sum_u scale[u] * tanh(q[i,u] + k[j,u])
  attn = softmax(scores, axis=-1)
  ctx = attn @ value

Sharding: the B*TQ = 2048 query rows split into 8 chunks of 256; core c
handles batch c//4, query rows (c%4)*256 .. +256.  Each core gets its
query chunk plus the full value/W1/W2/scale (data-parallel, no
collectives).

Per-core dataflow (U = 128 = partition count):
  - load query/value/W tiles (DMAs spread over three queues), convert to
    fp16 on DVE, PE-transpose the fp16 copies to get d-major layouts
    (fp16 keeps the PE off its 2-pass fp32 LOW_HIGH weight-load path;
    fp16 rounding of the projection inputs costs ~5e-4 relative)
  - kprojT [U, TV] and qprojT [U, QCH] via fp16 PE matmuls (fp32 psum)
  - per query i: ACT computes t = tanh(kprojT + qprojT[:, i]) in one
    [128, 1024] instruction (per-partition bias), output cast to fp16;
    8 PE matmuls with t as the stationary operand and scale [128,1]
    moving produce the score column for each 128-key tile
  - per 64-query half-tile: PE-transpose scoresT back (fp32, separate
    psum/sbuf tiles per half so the first half's softmax prep overlaps
    the tanh stream), batched softmax (DVE max -> ACT exp with
    accumulator -> DVE reciprocal -> DVE scale), fp16 attn copy for the
    context matmul against fp16 value tiles.  Both qtiles' tanh/score
    loops are emitted before any softmax so the ACT stream never stalls.
"""

from contextlib import ExitStack

import numpy as np

from concourse import bacc, bass, masks, mybir
import concourse.tile as tile
from concourse.bass_utils import run_bass_kernel_spmd

F32 = mybir.dt.float32
F16 = mybir.dt.float16
AF = mybir.ActivationFunctionType
ALU = mybir.AluOpType
AX = mybir.AxisListType

B, TQ, TV, D, U = 2, 1024, 1024, 512, 128
NCORES = 8
QCH = (B * TQ) // NCORES  # 256 query rows per core
NQT = QCH // 128          # 2 query tiles per core
NVT = TV // 128           # 8 value tiles
NDT = D // 128            # 4 d tiles


def build_program() -> bass.Bass:
    nc = bacc.Bacc(None, target_bir_lowering=False)
    # fp16 (and pre-transposed where needed) inputs, prepared host-side in
    # make_in_maps; the device consumes only these layouts.
    v16_in = nc.declare_dram_parameter("v16", [TV, D], F16, isOutput=False)
    vT16_in = nc.declare_dram_parameter("vT16", [D, TV], F16, isOutput=False)
    qT16_in = nc.declare_dram_parameter("qT16", [D, QCH], F16, isOutput=False)
    w1_in = nc.declare_dram_parameter("w1_16", [D, U], F16, isOutput=False)
    w2_in = nc.declare_dram_parameter("w2_16", [D, U], F16, isOutput=False)
    s_in = nc.declare_dram_parameter("s16", [U, 1], F16, isOutput=False)
    attn_out = nc.declare_dram_parameter("attn", [QCH, TV], F32, isOutput=True)
    ctx_out = nc.declare_dram_parameter("ctx", [QCH, D], F32, isOutput=True)

    with tile.TileContext(nc) as tc, ExitStack() as ctx:
        const = ctx.enter_context(tc.tile_pool(name="const", bufs=1))
        sb = ctx.enter_context(tc.tile_pool(name="sb", bufs=1))
        tpool = ctx.enter_context(tc.tile_pool(name="tpool", bufs=4))
        qt_pool = ctx.enter_context(tc.tile_pool(name="qt_pool", bufs=2))
        half_pool = ctx.enter_context(tc.tile_pool(name="half_pool", bufs=4))
        small = ctx.enter_context(tc.tile_pool(name="small", bufs=8))
        # PSUM: scoresT half tiles are 1 bank each, 4 alive at once;
        # everything else shares one-bank slots.
        ps_sc = ctx.enter_context(tc.tile_pool(name="ps_sc", bufs=2, space="PSUM"))
        ps_misc = ctx.enter_context(tc.tile_pool(name="ps_misc", bufs=2, space="PSUM"))
        ps_tr = ctx.enter_context(tc.tile_pool(name="ps_tr", bufs=4, space="PSUM"))

        identity = const.tile([128, 128], F32)
        masks.make_identity(nc, identity[:])
        identity16 = const.tile([128, 128], F16)
        masks.make_identity(nc, identity16[:])

        s16_sb = const.tile([U, 1], F16)
        nc.scalar.dma_start(out=s16_sb[:], in_=s_in[:])

        # weight + transposed-operand loads gate the projections; the
        # non-transposed value copy (ctx matmul operand) is only needed at
        # the end of the stream, so it loads last on the sync queue.
        w2_16_sb = sb.tile([128, NDT, U], F16)
        for d in range(NDT):
            nc.gpsimd.dma_start(out=w2_16_sb[:, d, :], in_=w2_in[d * 128:(d + 1) * 128, :])
        valueT16_sb = sb.tile([128, NDT, TV], F16)
        for d in range(NDT):
            nc.gpsimd.dma_start(out=valueT16_sb[:, d, :], in_=vT16_in[d * 128:(d + 1) * 128, :])
        w1_16_sb = sb.tile([128, NDT, U], F16)
        for d in range(NDT):
            nc.scalar.dma_start(out=w1_16_sb[:, d, :], in_=w1_in[d * 128:(d + 1) * 128, :])
        queryT16_sb = sb.tile([128, NDT, QCH], F16)
        for d in range(NDT):
            nc.scalar.dma_start(out=queryT16_sb[:, d, :], in_=qT16_in[d * 128:(d + 1) * 128, :])
        value16_sb = sb.tile([128, NVT, D], F16)
        for t in range(NVT):
            nc.sync.dma_start(out=value16_sb[:, t, :], in_=v16_in[t * 128:(t + 1) * 128, :])

        # qprojT[u, i] = sum_d W1[d, u] * query[i, d]   (fp32 accumulate)
        qprojT_sb = sb.tile([U, QCH], F32)
        psq = ps_misc.tile([U, QCH], F32, tag="ps_misc")
        for d in range(NDT):
            nc.tensor.matmul(psq[:], w1_16_sb[:, d, :], queryT16_sb[:, d, :],
                             start=(d == 0), stop=(d == NDT - 1))
        nc.vector.tensor_copy(qprojT_sb[:], psq[:])

        # kprojT[u, j] = sum_d W2[d, u] * value[j, d]
        kprojT_sb = sb.tile([U, TV], F32)
        for h in range(2):
            psk = ps_misc.tile([U, 512], F32, tag="ps_misc")
            for d in range(NDT):
                nc.tensor.matmul(psk[:], w2_16_sb[:, d, :],
                                 valueT16_sb[:, d, h * 512:(h + 1) * 512],
                                 start=(d == 0), stop=(d == NDT - 1))
            nc.vector.tensor_copy(kprojT_sb[:, h * 512:(h + 1) * 512], psk[:])

        # ---- main stream, emitted per 64-query half-block with a 1-half
        # lag: PE executes its queue in order, so each half's softmax-prep
        # transposes are queued right after the NEXT half's score matmuls.
        # That way they run while ACT is still on the next half's tanh
        # stream, and each exp's dependency chain is already done when ACT
        # reaches it.  Only the last half's chain is exposed at the end.

        def emit_half_scores(qt, h):
            # sc_ps[m, vt, c] = scores[qt*128 + h*64 + c, vt*128 + m]
            sc_ps = ps_sc.tile([128, NVT, 64], F32, tag="sc_ps_half")
            for c in range(64):
                i = qt * 128 + h * 64 + c
                t_t = tpool.tile([U, TV], F16, tag="t_t")
                nc.scalar.activation(out=t_t[:], in_=kprojT_sb[:], func=AF.Tanh,
                                     bias=qprojT_sb[:, i:i + 1], scale=1.0)
                for vt in range(NVT):
                    nc.tensor.matmul(sc_ps[:, vt, c:c + 1],
                                     t_t[:, vt * 128:(vt + 1) * 128], s16_sb[:],
                                     start=True, stop=True)
            return sc_ps

        def emit_half_softmax(qt, h, sc_ps):
            row0 = qt * 128 + h * 64
            scoresT_sb = half_pool.tile([128, NVT, 64], F32, tag="scoresT")
            nc.vector.tensor_copy(scoresT_sb[:], sc_ps[:])
            scores_sb = half_pool.tile([64, TV], F32, tag="scores")
            for vt in range(NVT):
                pst = ps_tr.tile([64, 128], F32, tag="ps_tr")
                nc.tensor.transpose(pst[:], scoresT_sb[:, vt, :], identity[:])
                nc.vector.tensor_copy(scores_sb[:, vt * 128:(vt + 1) * 128], pst[:])

            neg_max = small.tile([64, 1], F32, tag="small")
            nc.vector.tensor_reduce(out=neg_max[:], in_=scores_sb[:],
                                    axis=AX.X, op=ALU.max, negate=True)
            exp_sb = half_pool.tile([64, TV], F32, tag="exp")
            sums = small.tile([64, 1], F32, tag="small")
            nc.scalar.activation(out=exp_sb[:], in_=scores_sb[:],
                                 func=AF.Exp, bias=neg_max[:],
                                 accum_out=sums[:])
            recip = small.tile([64, 1], F32, tag="small")
            nc.vector.reciprocal(recip[:], sums[:])
            attn_sb = half_pool.tile([64, TV], F32, tag="attn")
            nc.vector.tensor_scalar_mul(attn_sb[:], exp_sb[:], recip[:])
            nc.sync.dma_start(out=attn_out[row0:row0 + 64, :], in_=attn_sb[:])

            # fp16 attn copy for the context matmul
            attn16_sb = half_pool.tile([64, TV], F16, tag="attn16")
            nc.vector.tensor_scalar_mul(attn16_sb[:], exp_sb[:], recip[:])
            attnT16_sb = half_pool.tile([128, NVT, 64], F16, tag="attnT")
            for vt in range(NVT):
                pst = ps_tr.tile([128, 64], F16, tag="ps_tr")
                nc.tensor.transpose(pst[:], attn16_sb[:, vt * 128:(vt + 1) * 128],
                                    identity16[0:64, 0:64])
                nc.vector.tensor_copy(attnT16_sb[:, vt, :], pst[:])

            ctx_ps = ps_tr.tile([64, D], F32, tag="ps_tr")
            for vt in range(NVT):
                nc.tensor.matmul(ctx_ps[:], attnT16_sb[:, vt, :],
                                 value16_sb[:, vt, :],
                                 start=(vt == 0), stop=(vt == NVT - 1))
            ctx_sb = half_pool.tile([64, D], F32, tag="ctx")
            nc.vector.tensor_copy(ctx_sb[:], ctx_ps[:])
            nc.sync.dma_start(out=ctx_out[row0:row0 + 64, :], in_=ctx_sb[:])

        halves = [(qt, h) for qt in range(NQT) for h in range(2)]
        prev = None
        for qt, h in halves:
            sc_ps = emit_half_scores(qt, h)
            if prev is not None:
                emit_half_softmax(*prev)
            prev = (qt, h, sc_ps)
        emit_half_softmax(*prev)

    nc.finalize()
    return nc


_program_cache: dict[str, bass.Bass] = {}


def _get_program() -> bass.Bass:
    if "nc" not in _program_cache:
        _program_cache["nc"] = build_program()
    return _program_cache["nc"]


def make_in_maps(query, value, W1, W2, scale):
    in_maps = []
    for c in range(NCORES):
        b = c // (NCORES // B)
        qc = c % (NCORES // B)
        qch = np.asarray(query[b, qc * QCH:(qc + 1) * QCH, :], dtype=np.float32)
        vb = np.asarray(value[b], dtype=np.float32)
        in_maps.append({
            "v16": np.ascontiguousarray(vb.astype(np.float16)),
            "vT16": np.ascontiguousarray(vb.T.astype(np.float16)),
            "qT16": np.ascontiguousarray(qch.T.astype(np.float16)),
            "w1_16": np.ascontiguousarray(np.asarray(W1, np.float32).astype(np.float16)),
            "w2_16": np.ascontiguousarray(np.asarray(W2, np.float32).astype(np.float16)),
            "s16": np.ascontiguousarray(
                np.asarray(scale, np.float32).reshape(U, 1).astype(np.float16)),
        })
    return in_maps


def assemble(results):
    ctx_full = np.empty((B, TQ, D), dtype=np.float32)
    attn_full = np.empty((B, TQ, TV), dtype=np.float32)
    for c in range(NCORES):
        b = c // (NCORES // B)
        qc = c % (NCORES // B)
        ctx_full[b, qc * QCH:(qc + 1) * QCH, :] = results[c]["ctx"]
        attn_full[b, qc * QCH:(qc + 1) * QCH, :] = results[c]["attn"]
    return ctx_full, attn_full


def kernel(query, value, W1, W2, scale):
    nc = _get_program()
    in_maps = make_in_maps(query, value, W1, W2, scale)
    res = run_bass_kernel_spmd(nc, in_maps, list(range(NCORES))).results
    return assemble(res)
